# revision 1
# baseline (speedup 1.0000x reference)
"""Multi-head attention (RoPE + mask + softmax) Trainium2 Bass kernel.

Sharding: 8 cores = 2 batches x 4 head-groups. Core c handles batch c//4,
local heads 4*(c%4) .. +4 (tensor-parallel on heads; Wq/Wk/Wv column-sharded,
Wo row-sharded; per-core partial outputs summed on host).

All DRAM inputs are host-pre-tiled so every DMA is partition-contiguous
(~128 descriptors). Per-core pipeline (S=2048, 4 heads of dim 64):
  qhT/khT = (Wq_perm)^T @ q^T   [2x128, 2048] f32r   (PE, K=1024 accum)
  RoPE fused into psum eviction: t = psum*cos, u = psum*sin_signed (DVE),
    swap 32-row blocks of u via SBUF->SBUF DMA (gpsimd queue), add (DVE)
  vh = v @ Wv  [2048, 4*65] bf16 with ones column per head (PE + strided ACT evict)
  per (q-block 1024, head-pair, k-chunk, head): scoresT[k,q] (PE, K=64),
    exp(x/8) (ACT psum->bf16), mask-mul (DVE bf16),
    attn@V accumulate [65, 1024] (PE bf16; row 64 = softmax denominator)
  denominators per (qb, head-pair): reciprocal_approx_accurate on a [128, 16]
    reshape (DRAM bounce), PE K=1 ones-broadcast, DVE normalize -> outT f32r
  out_part = outT^T @ Wo  (PE, 4x K=64 accum) -> [2048, 1024] f32
"""
import sys
sys.path.insert(0, '/opt/trn_rl_repo')
import math
import numpy as np
import ml_dtypes

import concourse.bass as bass
import concourse.mybir as mybir
import concourse.tile as tile
from concourse import bacc
from concourse.bass_utils import run_bass_kernel_spmd

F32 = mybir.dt.float32
F32R = mybir.dt.float32r
BF16 = mybir.dt.bfloat16

S = 2048
DIM = 1024
HEAD_DIM = 64
N_CORES = 8
KC = DIM // 128          # 8 contraction chunks for projections
MT = S // 128            # 16 k-chunks in attention
QB = 1024                # q-block width
NQB = S // QB            # 2
ROPE_THETA = 10000.0

_BUILT = None


def build_bass():
    nc = bacc.Bacc("TRN2", target_bir_lowering=False, debug=False)

    qT = nc.dram_tensor("qT", [4, 128, KC, 512], F32R, kind="ExternalInput").ap()
    kT = nc.dram_tensor("kT", [4, 128, KC, 512], F32R, kind="ExternalInput").ap()
    vT = nc.dram_tensor("vT", [MT, 128, KC, 128], F32R, kind="ExternalInput").ap()
    wq = nc.dram_tensor("wq", [128, KC, 256], F32R, kind="ExternalInput").ap()
    wk = nc.dram_tensor("wk", [128, KC, 256], F32R, kind="ExternalInput").ap()
    wv = nc.dram_tensor("wv", [128, KC, 256], F32R, kind="ExternalInput").ap()
    wo = nc.dram_tensor("wo", [64, 4, DIM], F32R, kind="ExternalInput").ap()
    cosT = nc.dram_tensor("cosT", [128, S], F32, kind="ExternalInput").ap()
    sinT = nc.dram_tensor("sinT", [128, S], F32, kind="ExternalInput").ap()
    maskT = nc.dram_tensor("maskT", [128, MT, S], BF16, kind="ExternalInput").ap()
    ones64 = nc.dram_tensor("ones64", [1, 64], F32R, kind="ExternalInput").ap()
    out_part = nc.dram_tensor("out_part", [S, DIM], F32, kind="ExternalOutput").ap()

    with tile.TileContext(nc) as tc:
        with tc.tile_pool(name="persist", bufs=1) as persist, \
             tc.tile_pool(name="dram", bufs=1, space="DRAM") as dram, \
             tc.tile_pool(name="ps", bufs=4, space="PSUM") as ps:

            qhT = persist.tile([128, 2, S], F32R)     # [chunk-part, chunk, s]
            khT = persist.tile([128, 2, S], F32R)
            vh = persist.tile([128, MT, 4 * 65], BF16)
            outT = persist.tile([64, 4, S], F32R)
            wo_sb = persist.tile([64, 4, DIM], F32R)
            ones_sb = persist.tile([1, 64], F32R)
            dscr = dram.tile([8, QB], F32)
            dscr2 = dram.tile([8, QB], F32R)

            nc.sync.dma_start(out=wo_sb, in_=wo)
            nc.sync.dma_start(out=ones_sb, in_=ones64)
            # ones column for the denominator rows of vh
            nc.vector.memset(
                vh.rearrange("p m (h x) -> p m h x", x=65)[:, :, :, 64:65], 1.0)

            # ---------------- Phase 1+2: projections + RoPE ----------------
            with tc.tile_pool(name="proj", bufs=1) as projp, \
                 tc.tile_pool(name="xts", bufs=2) as xts, \
                 tc.tile_pool(name="rope", bufs=2) as rope:
                wq_sb = projp.tile([128, KC, 256], F32R)
                wk_sb = projp.tile([128, KC, 256], F32R)
                wv_sb = projp.tile([128, KC, 256], F32R)
                cos_sb = projp.tile([128, S], F32)
                sin_sb = projp.tile([128, S], F32)
                nc.sync.dma_start(out=wq_sb, in_=wq)
                nc.sync.dma_start(out=wk_sb, in_=wk)
                nc.sync.dma_start(out=wv_sb, in_=wv)
                nc.sync.dma_start(out=cos_sb, in_=cosT)
                nc.sync.dma_start(out=sin_sb, in_=sinT)

                # q/k projections with fused RoPE eviction
                for xdram, w_sb, dstT in ((qT, wq_sb, qhT), (kT, wk_sb, khT)):
                    for sblk in range(4):
                        x_sb = xts.tile([128, KC, 512], F32R, tag="xts")
                        nc.sync.dma_start(out=x_sb, in_=xdram[sblk])
                        ss = slice(sblk * 512, (sblk + 1) * 512)
                        for m in range(2):
                            psum = ps.tile([128, QB], F32, tag="ps")
                            for kc in range(KC):
                                nc.tensor.matmul(
                                    psum[:, 0:512],
                                    lhsT=w_sb[:, kc, m * 128:(m + 1) * 128],
                                    rhs=x_sb[:, kc, :],
                                    start=(kc == 0), stop=(kc == KC - 1))
                            t = rope.tile([128, 512], F32, tag="t")
                            u = rope.tile([128, 512], F32, tag="u")
                            nc.vector.tensor_mul(t, psum[:, 0:512], cos_sb[:, ss])
                            nc.vector.tensor_mul(u, psum[:, 0:512], sin_sb[:, ss])
                            us = rope.tile([128, 512], F32, tag="us")
                            for blk in range(4):
                                a, b2 = blk * 32, (blk ^ 1) * 32
                                nc.gpsimd.dma_start(out=us[a:a + 32, :],
                                                    in_=u[b2:b2 + 32, :])
                            nc.vector.tensor_add(dstT[:, m, ss], t, us)

                # v projection with strided bf16 eviction (+ ones cols preset)
                for sc in range(MT):
                    v_sb = xts.tile([128, KC, 128], F32R, tag="xts")
                    nc.sync.dma_start(out=v_sb, in_=vT[sc])
                    psum = ps.tile([128, QB], F32, tag="ps")
                    for kc in range(KC):
                        nc.tensor.matmul(
                            psum[:, 0:256], lhsT=v_sb[:, kc, :], rhs=wv_sb[:, kc, :],
                            start=(kc == 0), stop=(kc == KC - 1))
                    nc.scalar.copy(
                        vh[:, sc, :].rearrange("p (h x) -> p h x", x=65)[:, :, 0:64],
                        psum[:, 0:256].rearrange("p (h x) -> p h x", x=64))

            # ---------------- Phase 3: attention ----------------
            with tc.tile_pool(name="mask", bufs=1) as maskp, \
                 tc.tile_pool(name="attn", bufs=3) as attnp, \
                 tc.tile_pool(name="dn", bufs=2) as dnp:
                mk = maskp.tile([128, MT, S], BF16, tag="mask")
                for mq in range(4):
                    nc.sync.dma_start(out=mk[:, mq * 4:(mq + 1) * 4, :],
                                      in_=maskT[:, mq * 4:(mq + 1) * 4, :])
                stg = dnp.tile([128, QB], F32, tag="stg")
                for qb in range(NQB):
                    qs = slice(qb * QB, (qb + 1) * QB)
                    for hp in range(2):
                        avp = [ps.tile([128, QB], F32, tag="ps", name=f"avp{_i}")
                               for _i in range(2)]
                        for m in range(MT):
                            for h2 in range(2):
                                hb = slice(h2 * 64, (h2 + 1) * 64)
                                sps = ps.tile([128, QB], F32, tag="ps")
                                for q2 in range(2):
                                    q5 = slice(q2 * 512, (q2 + 1) * 512)
                                    nc.tensor.matmul(
                                        sps[:, q5],
                                        lhsT=khT[hb, hp, m * 128:(m + 1) * 128],
                                        rhs=qhT[hb, hp, qb * QB + q2 * 512:
                                                qb * QB + (q2 + 1) * 512],
                                        start=True, stop=True)
                                at = attnp.tile([128, QB], BF16, tag="at")
                                nc.scalar.activation(
                                    at, sps, mybir.ActivationFunctionType.Exp,
                                    scale=1.0 / math.sqrt(HEAD_DIM))
                                atm = attnp.tile([128, QB], BF16, tag="atm")
                                nc.vector.tensor_mul(atm, at, mk[:, m, qs])
                                h = 2 * hp + h2
                                for q2 in range(2):
                                    q5 = slice(q2 * 512, (q2 + 1) * 512)
                                    nc.tensor.matmul(
                                        avp[h2][0:65, q5],
                                        lhsT=vh[:, m, h * 65:(h + 1) * 65],
                                        rhs=atm[:, q5],
                                        start=(m == 0), stop=(m == MT - 1))
                        # evict + normalize this (qb, head-pair) right away
                        for h2 in range(2):
                            h = 2 * hp + h2
                            unit = qb * 4 + hp * 2 + h2
                            nc.vector.tensor_copy(outT[0:64, h, qs], avp[h2][0:64, :])
                            nc.scalar.copy(stg[64:65, :], avp[h2][64:65, :])
                            nc.sync.dma_start(out=dscr[unit, :], in_=stg[64:65, :])
                        u0 = qb * 4 + hp * 2
                        rin = dnp.tile([128, 2, 8], F32, tag="rin")
                        nc.sync.dma_start(
                            out=rin,
                            in_=dscr[u0:u0 + 2].rearrange("u (p f) -> p u f", p=128))
                        r32 = dnp.tile([128, 2, 8], F32, tag="r32")
                        scr = dnp.tile([128, 2, 8], F32, tag="scr")
                        nc.vector.reciprocal_approx_accurate(r32, rin, scr)
                        rr = dnp.tile([128, 2, 8], F32R, tag="rr")
                        nc.vector.tensor_copy(rr, r32)
                        nc.sync.dma_start(
                            out=dscr2[u0:u0 + 2].rearrange("u (p f) -> p u f", p=128),
                            in_=rr)
                        for h2 in range(2):
                            h = 2 * hp + h2
                            unit = u0 + h2
                            rdn = dnp.tile([1, QB], F32R, tag="rdn")
                            nc.sync.dma_start(out=rdn, in_=dscr2[unit:unit + 1, :])
                            pbc = ps.tile([128, QB], F32, tag="ps")
                            for q2 in range(2):
                                q5 = slice(q2 * 512, (q2 + 1) * 512)
                                nc.tensor.matmul(pbc[0:64, q5], lhsT=ones_sb,
                                                 rhs=rdn[:, q5], start=True, stop=True)
                            nc.vector.tensor_mul(outT[0:64, h, qs],
                                                 outT[0:64, h, qs], pbc[0:64, :])

            # ---------------- Phase 5: output projection ----------------
            with tc.tile_pool(name="outp", bufs=3) as outp:
                for sc in range(MT):
                    wps = ps.tile([128, QB], F32, tag="ps")
                    for nb in range(2):
                        n5 = slice(nb * 512, (nb + 1) * 512)
                        for h in range(4):
                            nc.tensor.matmul(
                                wps[:, n5],
                                lhsT=outT[0:64, h, sc * 128:(sc + 1) * 128],
                                rhs=wo_sb[0:64, h, n5],
                                start=(h == 0), stop=(h == 3))
                    co = outp.tile([128, DIM], F32, tag="co")
                    nc.scalar.copy(co, wps)
                    nc.sync.dma_start(out=out_part[sc * 128:(sc + 1) * 128, :], in_=co)

    nc.compile()
    return nc


def _rope_perm_cols():
    """Column permutation of the 256-wide W slice for one core's 4 heads.

    Chunk c (0,1) holds local heads 2c, 2c+1 as rows
    [hA_even(32) | hA_odd(32) | hB_even(32) | hB_odd(32)].
    """
    cols = []
    for c in range(2):
        for j2 in range(2):          # which head within the chunk
            head = 2 * c + j2
            for blk in range(2):     # 0: even dims, 1: odd dims
                for i in range(32):
                    cols.append(head * 64 + 2 * i + blk)
    return np.array(cols)


def _cos_sin_tables():
    inv_freq = 1.0 / (ROPE_THETA ** (np.arange(0, HEAD_DIM, 2, dtype=np.float64)
                                     / HEAD_DIM))          # [32]
    ang = np.arange(S, dtype=np.float64)[None, :] * inv_freq[:, None]  # [32, S]
    cos32 = np.cos(ang).astype(np.float32)
    sin32 = np.sin(ang).astype(np.float32)
    cosT = np.tile(cos32, (4, 1))                           # [128, S]
    # sign: +sin at even-dim rows (blocks 0, 2), -sin at odd-dim rows (1, 3)
    sinT = np.concatenate([sin32, -sin32, sin32, -sin32], axis=0)
    return np.ascontiguousarray(cosT), np.ascontiguousarray(sinT)


def _tile_xT(xT):
    # [1024, 2048] -> [4 sblk, 128 part, 8 kc, 512]
    return np.ascontiguousarray(
        xT.reshape(KC, 128, 4, 512).transpose(2, 1, 0, 3))


def _tile_vT(vT):
    # [1024, 2048] -> [16 sc, 128 part, 8 kc, 128]
    return np.ascontiguousarray(
        vT.reshape(KC, 128, MT, 128).transpose(2, 1, 0, 3))


def _tile_w(w):
    # [1024, 256] -> [128, 8, 256]
    return np.ascontiguousarray(w.reshape(KC, 128, 256).transpose(1, 0, 2))


def _tile_mask(maskT_bf16):
    # [2048, 2048] -> [128, 16 m, 2048]
    return np.ascontiguousarray(
        maskT_bf16.reshape(MT, 128, S).transpose(1, 0, 2))


def kernel(q, k, v, mask, Wq, Wk, Wv, Wo, bo):
    global _BUILT
    if _BUILT is None:
        _BUILT = build_bass()
    nc = _BUILT

    q = np.asarray(q, np.float32)
    k = np.asarray(k, np.float32)
    v = np.asarray(v, np.float32)
    Wq = np.asarray(Wq, np.float32)
    Wk = np.asarray(Wk, np.float32)
    Wv = np.asarray(Wv, np.float32)
    Wo = np.asarray(Wo, np.float32)
    bo = np.asarray(bo, np.float32)
    mask = np.asarray(mask)

    cosT, sinT = _cos_sin_tables()
    ones64 = np.ones((1, 64), np.float32)
    perm = _rope_perm_cols()
    qTb = [_tile_xT(q[b].T) for b in range(2)]
    kTb = [_tile_xT(k[b].T) for b in range(2)]
    vTb = [_tile_vT(v[b].T) for b in range(2)]
    maskTb = [_tile_mask(mask[b, 0].T.astype(ml_dtypes.bfloat16)) for b in range(2)]

    in_maps = []
    for c in range(N_CORES):
        b = c // 4
        head_base = (c % 4) * 4
        cols = slice(head_base * 64, head_base * 64 + 256)
        in_maps.append({
            "qT": qTb[b], "kT": kTb[b], "vT": vTb[b],
            "wq": _tile_w(Wq[:, cols][:, perm]),
            "wk": _tile_w(Wk[:, cols][:, perm]),
            "wv": _tile_w(Wv[:, cols]),
            "wo": np.ascontiguousarray(
                Wo[cols, :].reshape(4, 64, DIM).transpose(1, 0, 2)),
            "cosT": cosT, "sinT": sinT,
            "maskT": maskTb[b], "ones64": ones64,
        })

    kernel._last_in_maps = in_maps
    res = run_bass_kernel_spmd(nc, in_maps, core_ids=list(range(N_CORES)))
    out = np.zeros((2, S, DIM), np.float32)
    for c in range(N_CORES):
        out[c // 4] += res.results[c]["out_part"]
    out += bo[None, None, :]
    return out



# revision 2
# speedup vs baseline: 1.0295x; 1.0295x over previous
"""Multi-head attention (RoPE + mask + softmax) Trainium2 Bass kernel, v3.

Sharding: 8 cores = 2 batches x 4 head-groups. Core c handles batch c//4,
local heads 4*(c%4) .. +4 (tensor-parallel on heads; Wq/Wk/Wv column-sharded,
Wo row-sharded; per-core partial outputs summed on host).

All matmul operands bf16. The attention main loop is software-pipelined:
attn@V for step i issues two steps after its scores matmul, so the PE never
waits on the ACT exp / DVE mask chain, stays busy, and keeps the HAM clock
gate at full rate. Denominators ride as ones-columns in vh, are evicted by
DVE at partition 64, partition-scattered by DMA into a [8,1024] tile, and
inverted by one reciprocal_approx_fast. Head pairs pack to 128 partitions so
the output projection contracts K=128.
"""
import sys
sys.path.insert(0, '/opt/trn_rl_repo')
import math
import numpy as np
import ml_dtypes

import concourse.bass as bass
import concourse.mybir as mybir
import concourse.tile as tile
from concourse import bacc
from concourse.bass_utils import run_bass_kernel_spmd

F32 = mybir.dt.float32
BF16 = mybir.dt.bfloat16

S = 2048
DIM = 1024
HEAD_DIM = 64
N_CORES = 8
KC = DIM // 128          # 8 contraction chunks for projections
MT = S // 128            # 16 k-chunks in attention
QB = 1024                # q-block width
NQB = S // QB            # 2
ROPE_THETA = 10000.0

_BUILT = None


def build_bass():
    nc = bacc.Bacc("TRN2", target_bir_lowering=False, debug=False)

    qT = nc.dram_tensor("qT", [4, 128, KC, 512], BF16, kind="ExternalInput").ap()
    kT = nc.dram_tensor("kT", [4, 128, KC, 512], BF16, kind="ExternalInput").ap()
    vT = nc.dram_tensor("vT", [MT, 128, KC, 128], BF16, kind="ExternalInput").ap()
    wq = nc.dram_tensor("wq", [128, KC, 256], BF16, kind="ExternalInput").ap()
    wk = nc.dram_tensor("wk", [128, KC, 256], BF16, kind="ExternalInput").ap()
    wv = nc.dram_tensor("wv", [128, KC, 256], BF16, kind="ExternalInput").ap()
    wo = nc.dram_tensor("wo", [128, 2, DIM], BF16, kind="ExternalInput").ap()
    cosT = nc.dram_tensor("cosT", [128, S], BF16, kind="ExternalInput").ap()
    sinT = nc.dram_tensor("sinT", [128, S], BF16, kind="ExternalInput").ap()
    maskT = nc.dram_tensor("maskT", [128, MT, S], BF16, kind="ExternalInput").ap()
    ones64 = nc.dram_tensor("ones64", [1, 64], BF16, kind="ExternalInput").ap()
    out_part = nc.dram_tensor("out_part", [S, DIM], BF16, kind="ExternalOutput").ap()

    with tile.TileContext(nc) as tc:
        with tc.tile_pool(name="persist", bufs=1) as persist, \
             tc.tile_pool(name="ps", bufs=2, space="PSUM") as ps, \
             tc.tile_pool(name="av", bufs=2, space="PSUM") as av:

            # zero-padded to full 128-contraction / 128-stationary shapes so
            # the attention matmuls register as full-array activity (HAM)
            qhT = persist.tile([128, 2, 2, S], BF16)  # [pad(h2), h2, hp, s]
            khT = persist.tile([128, 2, 2, S], BF16)
            vh = persist.tile([128, MT, 4 * 128], BF16)
            outT = persist.tile([128, 2, S], BF16)    # packed head pairs
            wo_sb = persist.tile([128, 2, DIM], BF16)
            ones_sb = persist.tile([1, 64], BF16)
            cos_sb = persist.tile([128, S], BF16)
            sin_sb = persist.tile([128, S], BF16)
            mk = persist.tile([128, MT, S], BF16)
            den_stg = persist.tile([128, 1, QB], F32)  # row 64 staging
            den_sb = persist.tile([8, QB], F32)
            den_r = persist.tile([8, QB], F32)
            den_bf = persist.tile([8, QB], BF16)
            den_bc = persist.tile([1, 2, QB], BF16)    # per-hp broadcast rows

            nc.gpsimd.dma_start(out=cos_sb, in_=cosT)
            nc.gpsimd.dma_start(out=sin_sb, in_=sinT)
            # per-head 128-col block: [data 64 | one | zeros 63]
            vh4 = vh.rearrange("p m (h x) -> p m h x", x=128)
            nc.vector.memset(vh4[:, :, :, 64:65], 1.0)
            nc.vector.memset(vh4[:, :, :, 65:128], 0.0)
            nc.vector.memset(den_sb, 1.0)
            # zero the dead halves of the padded q/k head tensors
            nc.vector.memset(qhT[64:128, 0, :, :], 0.0)
            nc.vector.memset(qhT[0:64, 1, :, :], 0.0)
            nc.vector.memset(khT[64:128, 0, :, :], 0.0)
            nc.vector.memset(khT[0:64, 1, :, :], 0.0)

            # ---------------- Phase 1: projections + RoPE ----------------
            with tc.tile_pool(name="proj", bufs=1) as projp, \
                 tc.tile_pool(name="xts", bufs=3) as xts, \
                 tc.tile_pool(name="rope", bufs=2) as rope:
                wq_sb = projp.tile([128, KC, 256], BF16)
                wk_sb = projp.tile([128, KC, 256], BF16)
                wv_sb = projp.tile([128, KC, 256], BF16)
                nc.sync.dma_start(out=wk_sb, in_=wk)
                nc.scalar.dma_start(out=wq_sb, in_=wq)
                nc.scalar.dma_start(out=wv_sb, in_=wv)

                # k then q projections with fused RoPE eviction; sblk
                # pairs; psums alternate between the two PSUM rings and the
                # rope adds trail two slices behind the muls so the DVE
                # stream never blocks on the swap-DMA latency
                rope_pend = []

                def rope_flush(n):
                    while len(rope_pend) > n:
                        dstT_, hp_, ss_, t_, us_ = rope_pend.pop(0)
                        nc.vector.tensor_add(dstT_[0:64, 0, hp_, ss_],
                                             t_[0:64, :], us_[0:64, :])
                        nc.vector.tensor_add(dstT_[64:128, 1, hp_, ss_],
                                             t_[64:128, :], us_[64:128, :])

                pidx = 0
                for xdram, w_sb, dstT in ((kT, wk_sb, khT), (qT, wq_sb, qhT)):
                    for pair in range(2):
                        xa = xts.tile([128, KC, 512], BF16, tag="xts")
                        xb = xts.tile([128, KC, 512], BF16, tag="xts")
                        dq = nc.sync if dstT is khT else nc.scalar
                        dq.dma_start(out=xa, in_=xdram[2 * pair])
                        dq.dma_start(out=xb, in_=xdram[2 * pair + 1])
                        if dstT is qhT and pair == 0:
                            nc.scalar.dma_start(out=mk[:, 0:4, :],
                                                in_=maskT[:, 0:4, :])
                        for hp in range(2):
                            pool = ps if pidx % 2 == 0 else av
                            psum = pool.tile([128, QB], F32,
                                             tag=("ps" if pool is ps else "av"))
                            pidx += 1
                            for half, xs in ((0, xa), (1, xb)):
                                h5 = slice(half * 512, (half + 1) * 512)
                                for kc in range(KC):
                                    nc.tensor.matmul(
                                        psum[:, h5],
                                        lhsT=w_sb[:, kc, hp * 128:(hp + 1) * 128],
                                        rhs=xs[:, kc, :],
                                        start=(kc == 0), stop=(kc == KC - 1))
                            # rope eviction in 512-wide slices
                            for half in range(2):
                                h5 = slice(half * 512, (half + 1) * 512)
                                ss = slice(pair * 1024 + half * 512,
                                           pair * 1024 + (half + 1) * 512)
                                t = rope.tile([128, 512], F32, tag="t")
                                u = rope.tile([128, 512], F32, tag="u")
                                nc.vector.tensor_mul(t, psum[:, h5],
                                                     cos_sb[:, ss])
                                nc.vector.tensor_mul(u, psum[:, h5],
                                                     sin_sb[:, ss])
                                us = rope.tile([128, 512], F32, tag="us")
                                for blk in range(4):
                                    a, b2 = blk * 32, (blk ^ 1) * 32
                                    nc.gpsimd.dma_start(out=us[a:a + 32, :],
                                                        in_=u[b2:b2 + 32, :])
                                rope_pend.append((dstT, hp, ss, t, us))
                                rope_flush(1)
                rope_flush(0)

                # v projection: 4 s-chunks per psum tile, one batched evict
                for sq in range(4):
                    pool = ps if sq % 2 == 0 else av
                    vps = pool.tile([128, QB], F32,
                                    tag=("ps" if pool is ps else "av"))
                    for sc4 in range(4):
                        sc = sq * 4 + sc4
                        v_sb = xts.tile([128, KC, 128], BF16, tag="vts")
                        nc.sync.dma_start(out=v_sb, in_=vT[sc])
                        c2 = slice(sc4 * 256, (sc4 + 1) * 256)
                        for kc in range(KC):
                            nc.tensor.matmul(
                                vps[:, c2], lhsT=v_sb[:, kc, :],
                                rhs=wv_sb[:, kc, :],
                                start=(kc == 0), stop=(kc == KC - 1))
                    nc.vector.tensor_copy(
                        vh[:, sq * 4:(sq + 1) * 4, :].rearrange(
                            "p m (h x) -> p m h x", x=128)[:, :, :, 0:64],
                        vps.rearrange("p (m h x) -> p m h x", m=4, h=4))
                nc.sync.dma_start(out=wo_sb, in_=wo)
                nc.sync.dma_start(out=ones_sb, in_=ones64)
                for mq in range(1, 4):
                    nc.scalar.dma_start(out=mk[:, mq * 4:(mq + 1) * 4, :],
                                        in_=maskT[:, mq * 4:(mq + 1) * 4, :])

            # ---------------- Phase 2: attention (pipelined) -------------
            with tc.tile_pool(name="norm", bufs=8) as normp:
              with tc.tile_pool(name="attn", bufs=4) as attnp:
                group_avp = {}
                pb_tiles = {}

                def emit_front(qb, hp, m, h2):
                    qs = slice(qb * QB, (qb + 1) * QB)
                    sps = ps.tile([128, QB], F32, tag="ps")
                    for q2 in range(2):
                        q5 = slice(q2 * 512, (q2 + 1) * 512)
                        nc.tensor.matmul(
                            sps[:, q5],
                            lhsT=khT[:, h2, hp, m * 128:(m + 1) * 128],
                            rhs=qhT[:, h2, hp, qb * QB + q2 * 512:
                                    qb * QB + (q2 + 1) * 512],
                            start=True, stop=True)
                    at = attnp.tile([128, QB], BF16, tag="at")
                    nc.scalar.activation(
                        at, sps, mybir.ActivationFunctionType.Exp,
                        scale=1.0 / math.sqrt(HEAD_DIM))
                    atm = attnp.tile([128, QB], BF16, tag="atm")
                    nc.vector.tensor_mul(atm, at, mk[:, m, qs])
                    return atm

                deferred = []

                def emit_attnv(qb, hp, m, h2, atm, step_i):
                    if (qb, hp) not in group_avp:
                        group_avp[(qb, hp)] = [
                            av.tile([128, QB], F32, tag="av", name=f"avp{_i}")
                            for _i in range(2)]
                    avp = group_avp[(qb, hp)]
                    vcol = (2 * hp + h2) * 128
                    for q2 in range(2):
                        q5 = slice(q2 * 512, (q2 + 1) * 512)
                        nc.tensor.matmul(
                            avp[h2][:, q5],
                            lhsT=vh[:, m, vcol:vcol + 128],
                            rhs=atm[:, q5],
                            start=(m == 0), stop=(m == MT - 1))
                    if m == MT - 1 and h2 == 1:
                        emit_evict(qb, hp, step_i)

                def emit_evict(qb, hp, step_i):
                    qs = slice(qb * QB, (qb + 1) * QB)
                    avp = group_avp.pop((qb, hp))
                    u0 = (qb * 2 + hp) * 2
                    nc.vector.tensor_copy(outT[0:64, hp, qs], avp[0][0:64, :])
                    nc.vector.tensor_copy(outT[64:128, hp, qs],
                                          avp[1][0:64, :])
                    for h2 in range(2):
                        nc.vector.tensor_copy(den_stg[64:65, 0, :],
                                              avp[h2][64:65, :])
                        nc.gpsimd.dma_start(out=den_sb[u0 + h2:u0 + h2 + 1, :],
                                            in_=den_stg[64:65, 0, :])
                    if hp == 1:
                        # give the denominator DVE/DMA chain ~7 steps of
                        # headroom before the pbc matmuls enter the PE stream
                        if qb == 0:
                            deferred.append((step_i + 10,
                                             lambda: emit_den_prep(0)))

                def emit_den_prep(qb):
                    nc.vector.reciprocal_approx_fast(den_r, den_sb)
                    nc.vector.tensor_copy(den_bf, den_r)
                    for hp in range(2):
                        pbs = []
                        for h2 in range(2):
                            u = qb * 4 + hp * 2 + h2
                            nc.gpsimd.dma_start(out=den_bc[0:1, h2, :],
                                                in_=den_bf[u:u + 1, :])
                        for h2 in range(2):
                            pb = normp.tile([128, QB], BF16, tag="pbc")
                            nc.gpsimd.partition_broadcast(pb,
                                                          den_bc[0:1, h2, :])
                            pbs.append(pb)
                        pb_tiles[(qb, hp)] = pbs

                def emit_norm_muls(qb):
                    qs = slice(qb * QB, (qb + 1) * QB)
                    for hp in range(2):
                        pb0, pb1 = pb_tiles.pop((qb, hp))
                        nc.vector.tensor_mul(outT[0:64, hp, qs],
                                             outT[0:64, hp, qs], pb0[0:64, :])
                        nc.vector.tensor_mul(outT[64:128, hp, qs],
                                             outT[64:128, hp, qs],
                                             pb1[64:128, :])

                steps = [(qb, hp, m, h2)
                         for qb in range(NQB) for hp in range(2)
                         for m in range(MT) for h2 in range(2)]
                pending = []
                for i, st in enumerate(steps):
                    atm = emit_front(*st)
                    pending.append((st, atm))
                    if len(pending) > 2:
                        (pst, patm) = pending.pop(0)
                        emit_attnv(*pst, patm, i)
                    while deferred and deferred[0][0] <= i:
                        deferred.pop(0)[1]()
                last_i = len(steps)
                for (pst, patm) in pending:
                    emit_attnv(*pst, patm, last_i)
                emit_den_prep(1)

              # ---------------- Phase 3: output projection --------------
              # normalize qb0, project its rows while the qb1 broadcast chain
              # finishes, then normalize qb1 and project the rest
              with tc.tile_pool(name="outp", bufs=3) as outp:
                def emit_outproj(sc):
                    pool = ps if sc % 2 == 0 else av
                    wps = pool.tile([128, QB], F32,
                                    tag=("ps" if pool is ps else "av"))
                    for nb in range(2):
                        n5 = slice(nb * 512, (nb + 1) * 512)
                        for hp in range(2):
                            nc.tensor.matmul(
                                wps[:, n5],
                                lhsT=outT[:, hp, sc * 128:(sc + 1) * 128],
                                rhs=wo_sb[:, hp, n5],
                                start=(hp == 0), stop=(hp == 1))
                    co = outp.tile([128, DIM], BF16, tag="co")
                    if sc % 2 == 0:
                        nc.scalar.copy(co, wps)
                    else:
                        nc.vector.tensor_copy(co, wps)
                    nc.sync.dma_start(out=out_part[sc * 128:(sc + 1) * 128, :],
                                      in_=co)

                emit_norm_muls(0)
                for sc in range(8):
                    emit_outproj(sc)
                emit_norm_muls(1)
                for sc in range(8, MT):
                    emit_outproj(sc)

    nc.compile()
    return nc


def _rope_perm_cols():
    """Column permutation of the 256-wide W slice for one core's 4 heads.

    Pair hp (0,1) holds local heads 2hp, 2hp+1 as psum rows
    [hA_even(32) | hA_odd(32) | hB_even(32) | hB_odd(32)].
    """
    cols = []
    for c in range(2):
        for j2 in range(2):          # which head within the pair
            head = 2 * c + j2
            for blk in range(2):     # 0: even dims, 1: odd dims
                for i in range(32):
                    cols.append(head * 64 + 2 * i + blk)
    return np.array(cols)


def _cos_sin_tables():
    inv_freq = 1.0 / (ROPE_THETA ** (np.arange(0, HEAD_DIM, 2, dtype=np.float64)
                                     / HEAD_DIM))          # [32]
    ang = np.arange(S, dtype=np.float64)[None, :] * inv_freq[:, None]  # [32, S]
    cos32 = np.cos(ang).astype(np.float32)
    sin32 = np.sin(ang).astype(np.float32)
    cosT = np.tile(cos32, (4, 1))                           # [128, S]
    # sign: +sin at even-dim rows (blocks 0, 2), -sin at odd-dim rows (1, 3)
    sinT = np.concatenate([sin32, -sin32, sin32, -sin32], axis=0)
    return (np.ascontiguousarray(cosT.astype(ml_dtypes.bfloat16)),
            np.ascontiguousarray(sinT.astype(ml_dtypes.bfloat16)))


def _bf16(x):
    return np.ascontiguousarray(x.astype(ml_dtypes.bfloat16))


def _tile_xT(xT):
    # [1024, 2048] -> [4 sblk, 128 part, 8 kc, 512]
    return np.ascontiguousarray(
        xT.reshape(KC, 128, 4, 512).transpose(2, 1, 0, 3))


def _tile_vT(vT_):
    # [1024, 2048] -> [16 sc, 128 part, 8 kc, 128]
    return np.ascontiguousarray(
        vT_.reshape(KC, 128, MT, 128).transpose(2, 1, 0, 3))


def _tile_w(w):
    # [1024, 256] -> [128, 8, 256]
    return np.ascontiguousarray(w.reshape(KC, 128, 256).transpose(1, 0, 2))


def _tile_mask(maskT_bf16):
    # [2048, 2048] -> [128, 16 m, 2048]
    return np.ascontiguousarray(
        maskT_bf16.reshape(MT, 128, S).transpose(1, 0, 2))


def kernel(q, k, v, mask, Wq, Wk, Wv, Wo, bo):
    global _BUILT
    if _BUILT is None:
        _BUILT = build_bass()
    nc = _BUILT

    q = np.asarray(q, np.float32)
    k = np.asarray(k, np.float32)
    v = np.asarray(v, np.float32)
    Wq = np.asarray(Wq, np.float32)
    Wk = np.asarray(Wk, np.float32)
    Wv = np.asarray(Wv, np.float32)
    Wo = np.asarray(Wo, np.float32)
    bo = np.asarray(bo, np.float32)
    mask = np.asarray(mask)

    cosT, sinT = _cos_sin_tables()
    ones64 = np.ones((1, 64), ml_dtypes.bfloat16)
    perm = _rope_perm_cols()
    qTb = [_tile_xT(_bf16(q[b].T)) for b in range(2)]
    kTb = [_tile_xT(_bf16(k[b].T)) for b in range(2)]
    vTb = [_tile_vT(_bf16(v[b].T)) for b in range(2)]
    maskTb = [_tile_mask(mask[b, 0].T.astype(ml_dtypes.bfloat16))
              for b in range(2)]

    in_maps = []
    for c in range(N_CORES):
        b = c // 4
        head_base = (c % 4) * 4
        cols = slice(head_base * 64, head_base * 64 + 256)
        # wo rows packed (parity*64+d, hp): local head 2hp+parity, dim d
        wo_c = Wo[cols, :].reshape(2, 2, 64, DIM).transpose(1, 2, 0, 3)
        in_maps.append({
            "qT": qTb[b], "kT": kTb[b], "vT": vTb[b],
            "wq": _tile_w(_bf16(Wq[:, cols][:, perm])),
            "wk": _tile_w(_bf16(Wk[:, cols][:, perm])),
            "wv": _tile_w(_bf16(Wv[:, cols])),
            "wo": _bf16(wo_c.reshape(128, 2, DIM)),
            "cosT": cosT, "sinT": sinT,
            "maskT": maskTb[b], "ones64": ones64,
        })

    kernel._last_in_maps = in_maps
    res = run_bass_kernel_spmd(nc, in_maps, core_ids=list(range(N_CORES)))
    out = np.zeros((2, S, DIM), np.float32)
    for c in range(N_CORES):
        out[c // 4] += np.asarray(res.results[c]["out_part"], np.float32)
    out += bo[None, None, :]
    return out
